# revision 26
# baseline (speedup 1.0000x reference)
"""Multi-head attention block (QKV proj + softmax attention + out proj) on 8
Trainium2 NeuronCores, data-parallel over the batch dimension (one batch
element per core).

Self-contained: hardcodes shapes for x [8, 1024, 768], qkv_w [768, 2304],
proj_w [768, 768], proj_b [768]; returns [8, 1024, 768] float32.

v2: all-bf16 datapath, host-side pre-layout so every bulk DMA is contiguous
per partition, head-pair-pipelined emission so ACT exp overlaps QKV matmuls,
PE warmup matmuls.
"""

import numpy as np

import concourse.bass as bass
import concourse.mybir as mybir
import concourse.tile as tile
from concourse import bacc

N_CORES = 8
N = 1024          # tokens per batch element
C = 768           # model dim
H = 12            # heads
HD = 64           # head dim
CT = C // 128     # 6 contraction tiles
TT = N // 128     # 8 token tiles
SCALE = HD ** -0.5

F32 = mybir.dt.float32
BF16 = mybir.dt.bfloat16


def _build():
    nc = bacc.Bacc("TRN2", target_bir_lowering=False, debug=False,
                   num_devices=N_CORES)
    # host pre-laid-out inputs (see make_in_maps):
    #   xt[p, tt, ct, j]  = x[tt*128+j, ct*128+p]      (x^T, token-tiled)
    #   wqk[p, ft, ct, j] = qkv_w[ct*128+p, ft*128+j]  (Q|K cols, ft-tiled)
    #   wv[p, ct, f]      = qkv_w[ct*128+p, 1536+f]
    #   pw[p, ct, f]      = proj_w[ct*128+p, f]
    xt = nc.dram_tensor("xt", [128, TT, CT, 128], BF16, kind="ExternalInput").ap()
    wqk = nc.dram_tensor("wqk", [128, 2 * CT, CT, 128], BF16, kind="ExternalInput").ap()
    wv = nc.dram_tensor("wv", [128, CT, C], BF16, kind="ExternalInput").ap()
    pw = nc.dram_tensor("pw", [128, CT, C], BF16, kind="ExternalInput").ap()
    proj_b = nc.dram_tensor("proj_b", [1, C], F32, kind="ExternalInput").ap()
    out = nc.dram_tensor("out", [N, C], F32, kind="ExternalOutput").ap()

    with tile.TileContext(nc) as tc:
        _emit(nc, tc, xt, wqk, wv, pw, proj_b, out)
    nc.compile()
    return nc


def _emit(nc, tc, xt, wqk, wv, pw, proj_b, out):
    from contextlib import ExitStack
    ctx = ExitStack()
    with ctx:
        wqk_pool = ctx.enter_context(tc.tile_pool(name="wqk", bufs=1))
        xt_pool = ctx.enter_context(tc.tile_pool(name="xt", bufs=1))
        w768_pool = ctx.enter_context(tc.tile_pool(name="w768", bufs=1))
        qk_pool = ctx.enter_context(tc.tile_pool(name="qk", bufs=1))
        vaug_pool = ctx.enter_context(tc.tile_pool(name="vaug", bufs=1))
        exps_pool = ctx.enter_context(tc.tile_pool(name="exps", bufs=8))
        misc_pool = ctx.enter_context(tc.tile_pool(name="misc", bufs=3))
        norm_pool = ctx.enter_context(tc.tile_pool(name="norm", bufs=2))
        const_pool = ctx.enter_context(tc.tile_pool(name="const", bufs=1))
        outsb_pool = ctx.enter_context(tc.tile_pool(name="outsb", bufs=2))
        dram_pool = ctx.enter_context(tc.tile_pool(name="drs", bufs=2, space="DRAM"))

        sc_ps = ctx.enter_context(tc.tile_pool(name="scps", bufs=2, space="PSUM"))
        av_ps = ctx.enter_context(tc.tile_pool(name="avps", bufs=2, space="PSUM"))

        # ---- PE warmup: keep HAM clock-gate busy from t=0 (no DMA deps) ----
        wrm = const_pool.tile([128, 128], BF16, tag="wrm")
        nc.vector.memset(wrm[:], 0.0)
        wps = sc_ps.tile([128, 512], F32, tag="scps")
        for i in range(48):
            nc.tensor.matmul(wps[:, 0:128], lhsT=wrm[:], rhs=wrm[:],
                             start=True, stop=True)

        # ---- loads: few large DMAs (each dma_start costs ~1.3us of queue
        # sequencer time, so batching matters more than fine-grained overlap).
        # wqk comes host-reordered as [Q0,K0,Q1,K1,...] so QK0's two blocks
        # are one leading DMA.
        Wqk = wqk_pool.tile([128, 2 * CT, CT, 128], BF16, tag="wqk")
        nc.sync.dma_start(Wqk[:, 0:2, :, :], wqk[:, 0:2, :, :])
        XT = xt_pool.tile([128, TT, CT, 128], BF16, tag="xt")
        nc.sync.dma_start(XT[:], xt[:, :, :, :])
        Wv = w768_pool.tile([128, CT, C], BF16, tag="w768")
        nc.sync.dma_start(Wv[:], wv[:, :, :])
        nc.sync.dma_start(Wqk[:, 2:2 * CT, :, :], wqk[:, 2:2 * CT, :, :])
        PW = w768_pool.tile([128, CT, C], BF16, tag="pw768")
        nc.sync.dma_start(PW[:], pw[:, :, :])
        pbb = const_pool.tile([128, C], F32, tag="pb")
        pb_src = proj_b[:, :]
        pb_bcast = bass.AP(tensor=pb_src.tensor, offset=pb_src.offset,
                           ap=[[0, 128]] + [list(a) for a in pb_src.ap[1:]])
        nc.sync.dma_start(pbb[:], pb_bcast)
        ones_st = const_pool.tile([128, 128], F32, tag="ones_st")
        nc.vector.memset(ones_st[:], 1.0)

        # V_AUG column HD is the all-ones softmax-denominator column: the AV
        # output's D row lands on (32-aligned) partition 64.
        V_AUG = vaug_pool.tile([128, TT, H, HD + 1], BF16, tag="vaug")
        nc.vector.tensor_copy(
            V_AUG[:, :, :, HD:HD + 1].rearrange("p t h one -> p (t h one)"),
            ones_st[:, 0:96])
        QT = qk_pool.tile([128, CT, N], BF16, tag="qt")
        # K stored zero-padded per head half: scores run as full-128
        # contraction (the zero rows annihilate the other head's Q rows), so
        # the PE never leaves 128x128 tiling mode (mode switches drain it).
        KTA = qk_pool.tile([128, CT, N], BF16, tag="kta")
        KTB = qk_pool.tile([128, CT, N], BF16, tag="ktb")
        # gpsimd: keep the (busier) DVE free for PSUM evacuation
        nc.gpsimd.memset(KTA[64:128, :, :].rearrange("p c n -> p (c n)"), 0.0)
        nc.gpsimd.memset(KTB[0:64, :, :].rearrange("p c n -> p (c n)"), 0.0)
        outT = wqk_pool.tile([128, CT, N], BF16, tag="outT")

        def emit_v(tt, vc):
            # V token-tile tt chunk vc: x@Wv -> V_AUG[:, tt, heads, 0:64]
            w0, wn, h0 = [(0, 512, 0), (512, 256, 8)][vc]
            ps = sc_ps.tile([128, 512], F32, tag="scps")
            for ct in range(CT):
                nc.tensor.matmul(
                    ps[:, :wn],
                    lhsT=XT[:, tt, ct, :],
                    rhs=Wv[:, ct, w0:w0 + wn],
                    start=(ct == 0), stop=(ct == CT - 1))
            nc.vector.tensor_copy(
                V_AUG[:, tt, h0:h0 + wn // HD, 0:HD],
                ps[:, :wn].rearrange("p (h d) -> p h d", d=HD))

        def emit_qk(hp, part):
            # part 0..3 = (Q,qc0) (Q,qc1) (K,qc0) (K,qc1) of head pair hp.
            # wqk blocks are host-interleaved: Q of hp at 2*hp, K at 2*hp+1.
            is_k = part // 2 == 1
            ft = 2 * hp + 1 if is_k else 2 * hp
            qc = part % 2
            ps = sc_ps.tile([128, 512], F32, tag="scps")
            for ct in range(CT):
                nc.tensor.matmul(
                    ps[:],
                    lhsT=Wqk[:, ft, ct, :],
                    rhs=XT[:, 4 * qc:4 * qc + 4, ct, :],
                    start=(ct == 0), stop=(ct == CT - 1))
            sl = slice(qc * 512, (qc + 1) * 512)
            if is_k:
                # scalar engine evacuates K (PSUM->SBUF cast): offloads DVE,
                # and these copies are never latency-critical
                nc.scalar.activation(KTA[0:64, hp, sl], ps[0:64, :],
                                     mybir.ActivationFunctionType.Copy)
                nc.scalar.activation(KTB[64:128, hp, sl], ps[64:128, :],
                                     mybir.ActivationFunctionType.Copy)
            else:
                nc.vector.tensor_copy(QT[:, hp, sl], ps[:])

        def emit_scores(hp, kt):
            psA = sc_ps.tile([128, 1024], F32, tag="scps")
            psB = sc_ps.tile([128, 1024], F32, tag="scps")
            for qc in range(2):
                nc.tensor.matmul(
                    psA[:, qc * 512:(qc + 1) * 512],
                    lhsT=KTA[:, hp, kt * 128:(kt + 1) * 128],
                    rhs=QT[:, hp, qc * 512:(qc + 1) * 512],
                    start=True, stop=True)
                nc.tensor.matmul(
                    psB[:, qc * 512:(qc + 1) * 512],
                    lhsT=KTB[:, hp, kt * 128:(kt + 1) * 128],
                    rhs=QT[:, hp, qc * 512:(qc + 1) * 512],
                    start=True, stop=True)
            eA = exps_pool.tile([128, 1024], BF16, tag="exps")
            eB = exps_pool.tile([128, 1024], BF16, tag="exps")
            nc.scalar.activation(eA[:], psA[:],
                                 mybir.ActivationFunctionType.Exp, scale=SCALE)
            nc.scalar.activation(eB[:], psB[:],
                                 mybir.ActivationFunctionType.Exp, scale=SCALE)
            return eA, eB

        av_tiles = {}

        def emit_av(hp, kt, eA, eB):
            if hp not in av_tiles:
                avA = av_ps.tile([HD + 1, 1024], F32, tag="avps")
                avB = av_ps.tile([HD + 1, 1024], F32, tag="avps")
                av_tiles[hp] = (avA, avB)
            avA, avB = av_tiles[hp]
            for qc in range(2):
                nc.tensor.matmul(
                    avA[:, qc * 512:(qc + 1) * 512],
                    lhsT=V_AUG[:, kt, 2 * hp, :],
                    rhs=eA[:, qc * 512:(qc + 1) * 512],
                    start=(kt == 0), stop=(kt == TT - 1))
                nc.tensor.matmul(
                    avB[:, qc * 512:(qc + 1) * 512],
                    lhsT=V_AUG[:, kt, 2 * hp + 1, :],
                    rhs=eB[:, qc * 512:(qc + 1) * 512],
                    start=(kt == 0), stop=(kt == TT - 1))

        def emit_norm(hp):
            # outT[poff.., hp, :] = V-rows * (1/D); av row 0 is D, rows
            # 1..HD the unnormalized AV. All on-chip: broadcast D across
            # partitions (GpSimd), reciprocal at 64 lanes (DVE), multiply.
            avA, avB = av_tiles.pop(hp)
            for av, poff in ((avA, 0), (avB, 64)):
                U = norm_pool.tile([HD + 1, 1024], F32, tag="U")
                nc.vector.tensor_copy(U[:], av[:])
                # HW partition_broadcast reads physical partition 0, so stage
                # the D row into a partition-0 tile first
                Drow = norm_pool.tile([1, 1024], F32, tag="Drow")
                nc.vector.tensor_copy(Drow[:], U[HD:HD + 1, :])
                bcD = norm_pool.tile([64, 1024], F32, tag="bcD")
                nc.gpsimd.partition_broadcast(bcD[:], Drow[:])
                bc = norm_pool.tile([64, 1024], F32, tag="bc")
                scr = norm_pool.tile([64, 1024], F32, tag="scr")
                nc.vector.reciprocal_approx_accurate(bc[:], bcD[:], scr[:])
                for qc in range(2):
                    nc.vector.tensor_mul(
                        outT[poff:poff + 64, hp, qc * 512:(qc + 1) * 512],
                        U[0:HD, qc * 512:(qc + 1) * 512],
                        bc[:, qc * 512:(qc + 1) * 512])

        # ---- head-pair-pipelined rounds ----
        # QK0 first (gated on the first two wqk DMAs + xt) so exp starts
        # ASAP; V fills hp0's rounds, next head pair's QK fills hp>=1.
        for part in range(4):
            emit_qk(0, part)
        avq = []           # (hp, kt, eA, eB) awaiting AV emission
        DELAY = 3
        pending_norms = []
        for hp in range(CT):
            for kt in range(TT):
                eA, eB = emit_scores(hp, kt)
                avq.append((hp, kt, eA, eB))
                if hp == 0:
                    emit_v(kt, 0)
                    emit_v(kt, 1)
                    if kt % 2 == 1:
                        emit_qk(1, kt // 2)
                elif hp < CT - 1 and kt % 2 == (hp & 1):
                    emit_qk(hp + 1, kt // 2)
                while len(avq) > DELAY:
                    h0, k0, e0, e1 = avq.pop(0)
                    emit_av(h0, k0, e0, e1)
                    if k0 == TT - 1:
                        emit_norm(h0)
        for h0, k0, e0, e1 in avq:
            emit_av(h0, k0, e0, e1)
            if k0 == TT - 1:
                pending_norms.append(h0)

        # ---- proj, split so ct 0..4 runs while the last head pair's norm
        # chain completes; only the ct=5 contribution trails it ----
        for h0 in pending_norms:
            emit_norm(h0)
        partials = []
        for tt in range(TT):
            pp = outsb_pool.tile([128, C], F32, tag="projp", bufs=TT)
            partials.append(pp)
            for nch in range(2):
                ps = sc_ps.tile([128, 384], F32, tag="scps")
                for ct in range(CT - 1):
                    nc.tensor.matmul(
                        ps[:],
                        lhsT=outT[:, ct, tt * 128:(tt + 1) * 128],
                        rhs=PW[:, ct, nch * 384:(nch + 1) * 384],
                        start=(ct == 0), stop=(ct == CT - 2))
                nc.vector.tensor_add(pp[:, nch * 384:(nch + 1) * 384], ps[:],
                                     pbb[:, nch * 384:(nch + 1) * 384])
        for tt in range(TT):
            osb = outsb_pool.tile([128, C], F32, tag="outsb")
            for nch in range(2):
                ps = sc_ps.tile([128, 384], F32, tag="scps")
                nc.tensor.matmul(
                    ps[:],
                    lhsT=outT[:, CT - 1, tt * 128:(tt + 1) * 128],
                    rhs=PW[:, CT - 1, nch * 384:(nch + 1) * 384],
                    start=True, stop=True)
                nc.vector.tensor_add(osb[:, nch * 384:(nch + 1) * 384], ps[:],
                                     partials[tt][:, nch * 384:(nch + 1) * 384])
            # scalar queue: sync still busy issuing bulk loads early on, and
            # the scalar queue is idle by proj time
            nc.scalar.dma_start(out[tt * 128:(tt + 1) * 128, :], osb[:])


_CACHE = {}


def _get_runner():
    """Build + compile once; return a callable(in_maps) -> list of out dicts."""
    if "runner" in _CACHE:
        return _CACHE["runner"]

    import jax
    from jax.experimental.shard_map import shard_map
    from jax.sharding import Mesh, PartitionSpec
    from concourse import bass2jax

    nc = _build()
    bass2jax.install_neuronx_cc_hook()

    partition_name = (nc.partition_id_tensor.name if nc.partition_id_tensor
                      else None)
    in_names, out_names, out_avals, zero_outs = [], [], [], []
    for alloc in nc.m.functions[0].allocations:
        if not isinstance(alloc, mybir.MemoryLocationSet):
            continue
        name = alloc.memorylocations[0].name
        if alloc.kind == "ExternalInput":
            if name != partition_name:
                in_names.append(name)
        elif alloc.kind == "ExternalOutput":
            out_names.append(name)
            shape = tuple(alloc.tensor_shape)
            dtype = mybir.dt.np(alloc.dtype)
            out_avals.append(jax.core.ShapedArray(shape, dtype))
            zero_outs.append(np.zeros(shape, dtype))
    n_params = len(in_names)
    n_outs = len(out_avals)
    all_in_names = list(in_names) + list(out_names)
    if partition_name is not None:
        all_in_names.append(partition_name)
    donate = tuple(range(n_params, n_params + n_outs))

    def _body(*args):
        operands = list(args)
        if partition_name is not None:
            operands.append(bass2jax.partition_id_tensor())
        outs = bass2jax._bass_exec_p.bind(
            *operands,
            out_avals=tuple(out_avals),
            in_names=tuple(all_in_names),
            out_names=tuple(out_names),
            lowering_input_output_aliases=(),
            sim_require_finite=True,
            sim_require_nnan=True,
            nc=nc,
        )
        return tuple(outs)

    devices = jax.devices()[:N_CORES]
    mesh = Mesh(np.asarray(devices), ("core",))
    in_specs = (PartitionSpec("core"),) * (n_params + n_outs)
    out_specs = (PartitionSpec("core"),) * n_outs
    sharded = jax.jit(
        shard_map(_body, mesh=mesh, in_specs=in_specs, out_specs=out_specs,
                  check_rep=False),
        donate_argnums=donate, keep_unused=True)

    def runner(in_maps):
        concat_in = [
            np.concatenate([np.asarray(m[name]) for m in in_maps], axis=0)
            for name in in_names
        ]
        concat_zeros = [
            np.zeros((N_CORES * z.shape[0], *z.shape[1:]), z.dtype)
            for z in zero_outs
        ]
        out_arrs = sharded(*concat_in, *concat_zeros)
        return [
            {name: np.asarray(out_arrs[i]).reshape(N_CORES, *out_avals[i].shape)[c]
             for i, name in enumerate(out_names)}
            for c in range(N_CORES)
        ]

    _CACHE["runner"] = runner
    _CACHE["nc"] = nc
    return runner


def make_in_maps(x, qkv_w, proj_w, proj_b):
    import ml_dtypes
    bf16 = ml_dtypes.bfloat16
    x = np.asarray(x, dtype=np.float32)
    qkv_w = np.asarray(qkv_w, dtype=np.float32)
    proj_w = np.asarray(proj_w, dtype=np.float32)
    pb = np.asarray(proj_b, dtype=np.float32).reshape(1, C)

    # wqk[p, i, ct, j] = qkv_w[ct*128+p, ft*128+j] with blocks interleaved
    # [Q0, K0, Q1, K1, ...] so the kernel's first DMA covers QK0 exactly
    ft_order = [b for hp in range(CT) for b in (hp, hp + CT)]
    wqk = np.ascontiguousarray(
        qkv_w[:, :2 * C].reshape(CT, 128, 2 * CT, 128).transpose(1, 2, 0, 3)
        [:, ft_order]).astype(bf16)
    # wv[p, ct, f] = qkv_w[ct*128+p, 1536+f]
    wv = np.ascontiguousarray(
        qkv_w[:, 2 * C:].reshape(CT, 128, C).transpose(1, 0, 2)).astype(bf16)
    # pw[p, ct, f] = proj_w[ct*128+p, f]
    pw = np.ascontiguousarray(
        proj_w.reshape(CT, 128, C).transpose(1, 0, 2)).astype(bf16)

    maps = []
    for b in range(N_CORES):
        # xt[p, tt, ct, j] = x[b, tt*128+j, ct*128+p]
        xtb = np.ascontiguousarray(
            np.asarray(x[b]).reshape(TT, 128, CT, 128).transpose(3, 0, 2, 1)
        ).astype(bf16)
        maps.append({"xt": xtb, "wqk": wqk, "wv": wv, "pw": pw, "proj_b": pb})
    return maps


def kernel(x, qkv_w, proj_w, proj_b):
    runner = _get_runner()
    results = runner(make_in_maps(x, qkv_w, proj_w, proj_b))
    return np.stack([results[b]["out"] for b in range(N_CORES)], axis=0)


# revision 27
# speedup vs baseline: 1.0504x; 1.0504x over previous
"""Multi-head attention block (QKV proj + softmax attention + out proj) on 8
Trainium2 NeuronCores, data-parallel over the batch dimension (one batch
element per core).

Self-contained: hardcodes shapes for x [8, 1024, 768], qkv_w [768, 2304],
proj_w [768, 768], proj_b [768]; returns [8, 1024, 768] float32.

v2: all-bf16 datapath, host-side pre-layout so every bulk DMA is contiguous
per partition, head-pair-pipelined emission so ACT exp overlaps QKV matmuls,
PE warmup matmuls.
"""

import numpy as np

import concourse.bass as bass
import concourse.mybir as mybir
import concourse.tile as tile
from concourse import bacc

N_CORES = 8
N = 1024          # tokens per batch element
C = 768           # model dim
H = 12            # heads
HD = 64           # head dim
CT = C // 128     # 6 contraction tiles
TT = N // 128     # 8 token tiles
SCALE = HD ** -0.5

F32 = mybir.dt.float32
BF16 = mybir.dt.bfloat16


def _build():
    nc = bacc.Bacc("TRN2", target_bir_lowering=False, debug=False,
                   num_devices=N_CORES)
    # host pre-laid-out inputs (see make_in_maps):
    #   xt[p, tt, ct, j]  = x[tt*128+j, ct*128+p]      (x^T, token-tiled)
    #   wqk[p, ft, ct, j] = qkv_w[ct*128+p, ft*128+j]  (Q|K cols, ft-tiled)
    #   wv[p, ct, f]      = qkv_w[ct*128+p, 1536+f]
    #   pw[p, ct, f]      = proj_w[ct*128+p, f]
    xt = nc.dram_tensor("xt", [128, TT, CT, 128], BF16, kind="ExternalInput").ap()
    wqk = nc.dram_tensor("wqk", [128, 2 * CT, CT, 128], BF16, kind="ExternalInput").ap()
    wv = nc.dram_tensor("wv", [128, CT, C], BF16, kind="ExternalInput").ap()
    pw = nc.dram_tensor("pw", [128, CT, C], BF16, kind="ExternalInput").ap()
    proj_b = nc.dram_tensor("proj_b", [1, C], F32, kind="ExternalInput").ap()
    out = nc.dram_tensor("out", [N, C], F32, kind="ExternalOutput").ap()

    with tile.TileContext(nc) as tc:
        _emit(nc, tc, xt, wqk, wv, pw, proj_b, out)
    nc.compile()
    return nc


def _emit(nc, tc, xt, wqk, wv, pw, proj_b, out):
    from contextlib import ExitStack
    ctx = ExitStack()
    with ctx:
        wqk_pool = ctx.enter_context(tc.tile_pool(name="wqk", bufs=1))
        xt_pool = ctx.enter_context(tc.tile_pool(name="xt", bufs=1))
        w768_pool = ctx.enter_context(tc.tile_pool(name="w768", bufs=1))
        qk_pool = ctx.enter_context(tc.tile_pool(name="qk", bufs=1))
        vaug_pool = ctx.enter_context(tc.tile_pool(name="vaug", bufs=1))
        exps_pool = ctx.enter_context(tc.tile_pool(name="exps", bufs=8))
        misc_pool = ctx.enter_context(tc.tile_pool(name="misc", bufs=3))
        norm_pool = ctx.enter_context(tc.tile_pool(name="norm", bufs=2))
        const_pool = ctx.enter_context(tc.tile_pool(name="const", bufs=1))
        outsb_pool = ctx.enter_context(tc.tile_pool(name="outsb", bufs=2))
        dram_pool = ctx.enter_context(tc.tile_pool(name="drs", bufs=2, space="DRAM"))

        sc_ps = ctx.enter_context(tc.tile_pool(name="scps", bufs=2, space="PSUM"))
        av_ps = ctx.enter_context(tc.tile_pool(name="avps", bufs=2, space="PSUM"))

        # ---- PE warmup: keep HAM clock-gate busy from t=0 (no DMA deps) ----
        wrm = const_pool.tile([128, 128], BF16, tag="wrm")
        nc.vector.memset(wrm[:], 0.0)
        wps = sc_ps.tile([128, 512], F32, tag="scps")
        for i in range(48):
            nc.tensor.matmul(wps[:, 0:128], lhsT=wrm[:], rhs=wrm[:],
                             start=True, stop=True)

        # ---- loads: few large DMAs (each dma_start costs ~1.3us of queue
        # sequencer time, so batching matters more than fine-grained overlap).
        # wqk comes host-reordered as [Q0,K0,Q1,K1,...] so QK0's two blocks
        # are one leading DMA.
        Wqk = wqk_pool.tile([128, 2 * CT, CT, 128], BF16, tag="wqk")
        nc.sync.dma_start(Wqk[:, 0:2, :, :], wqk[:, 0:2, :, :])
        XT = xt_pool.tile([128, TT, CT, 128], BF16, tag="xt")
        nc.sync.dma_start(XT[:], xt[:, :, :, :])
        Wv = w768_pool.tile([128, CT, C], BF16, tag="w768")
        nc.sync.dma_start(Wv[:], wv[:, :, :])
        nc.sync.dma_start(Wqk[:, 2:2 * CT, :, :], wqk[:, 2:2 * CT, :, :])
        PW = w768_pool.tile([128, CT, C], BF16, tag="pw768")
        nc.sync.dma_start(PW[:], pw[:, :, :])
        pbb = const_pool.tile([128, C], F32, tag="pb")
        pb_src = proj_b[:, :]
        pb_bcast = bass.AP(tensor=pb_src.tensor, offset=pb_src.offset,
                           ap=[[0, 128]] + [list(a) for a in pb_src.ap[1:]])
        nc.sync.dma_start(pbb[:], pb_bcast)
        ones_st = const_pool.tile([128, 128], F32, tag="ones_st")
        nc.vector.memset(ones_st[:], 1.0)

        # V_AUG column HD is the all-ones softmax-denominator column: the AV
        # output's D row lands on (32-aligned) partition 64.
        V_AUG = vaug_pool.tile([128, TT, H, HD + 1], BF16, tag="vaug")
        nc.vector.tensor_copy(
            V_AUG[:, :, :, HD:HD + 1].rearrange("p t h one -> p (t h one)"),
            ones_st[:, 0:96])
        QT = qk_pool.tile([128, CT, N], BF16, tag="qt")
        # K stored zero-padded per head half: scores run as full-128
        # contraction (the zero rows annihilate the other head's Q rows), so
        # the PE never leaves 128x128 tiling mode (mode switches drain it).
        KTA = qk_pool.tile([128, CT, N], BF16, tag="kta")
        KTB = qk_pool.tile([128, CT, N], BF16, tag="ktb")
        # gpsimd: keep the (busier) DVE free for PSUM evacuation
        nc.gpsimd.memset(KTA[64:128, :, :].rearrange("p c n -> p (c n)"), 0.0)
        nc.gpsimd.memset(KTB[0:64, :, :].rearrange("p c n -> p (c n)"), 0.0)
        outT = wqk_pool.tile([128, CT, N], BF16, tag="outT")

        def emit_v(tt, vc):
            # V token-tile tt chunk vc: x@Wv -> V_AUG[:, tt, heads, 0:64]
            w0, wn, h0 = [(0, 512, 0), (512, 256, 8)][vc]
            ps = sc_ps.tile([128, 512], F32, tag="scps")
            for ct in range(CT):
                nc.tensor.matmul(
                    ps[:, :wn],
                    lhsT=XT[:, tt, ct, :],
                    rhs=Wv[:, ct, w0:w0 + wn],
                    start=(ct == 0), stop=(ct == CT - 1))
            nc.vector.tensor_copy(
                V_AUG[:, tt, h0:h0 + wn // HD, 0:HD],
                ps[:, :wn].rearrange("p (h d) -> p h d", d=HD))

        def emit_qk(hp, part):
            # part 0..3 = (Q,qc0) (Q,qc1) (K,qc0) (K,qc1) of head pair hp.
            # wqk blocks are host-interleaved: Q of hp at 2*hp, K at 2*hp+1.
            is_k = part // 2 == 1
            ft = 2 * hp + 1 if is_k else 2 * hp
            qc = part % 2
            ps = sc_ps.tile([128, 512], F32, tag="scps")
            for ct in range(CT):
                nc.tensor.matmul(
                    ps[:],
                    lhsT=Wqk[:, ft, ct, :],
                    rhs=XT[:, 4 * qc:4 * qc + 4, ct, :],
                    start=(ct == 0), stop=(ct == CT - 1))
            sl = slice(qc * 512, (qc + 1) * 512)
            if is_k:
                # scalar engine evacuates K (PSUM->SBUF cast): offloads DVE,
                # and these copies are never latency-critical
                nc.scalar.activation(KTA[0:64, hp, sl], ps[0:64, :],
                                     mybir.ActivationFunctionType.Copy)
                nc.scalar.activation(KTB[64:128, hp, sl], ps[64:128, :],
                                     mybir.ActivationFunctionType.Copy)
            else:
                nc.vector.tensor_copy(QT[:, hp, sl], ps[:])

        def emit_scores(hp, kt):
            psA = sc_ps.tile([128, 1024], F32, tag="scps")
            psB = sc_ps.tile([128, 1024], F32, tag="scps")
            for qc in range(2):
                nc.tensor.matmul(
                    psA[:, qc * 512:(qc + 1) * 512],
                    lhsT=KTA[:, hp, kt * 128:(kt + 1) * 128],
                    rhs=QT[:, hp, qc * 512:(qc + 1) * 512],
                    start=True, stop=True)
                nc.tensor.matmul(
                    psB[:, qc * 512:(qc + 1) * 512],
                    lhsT=KTB[:, hp, kt * 128:(kt + 1) * 128],
                    rhs=QT[:, hp, qc * 512:(qc + 1) * 512],
                    start=True, stop=True)
            eA = exps_pool.tile([128, 1024], BF16, tag="exps")
            eB = exps_pool.tile([128, 1024], BF16, tag="exps")
            nc.scalar.activation(eA[:], psA[:],
                                 mybir.ActivationFunctionType.Exp, scale=SCALE)
            nc.scalar.activation(eB[:], psB[:],
                                 mybir.ActivationFunctionType.Exp, scale=SCALE)
            return eA, eB

        av_tiles = {}

        def emit_av(hp, kt, eA, eB):
            if hp not in av_tiles:
                avA = av_ps.tile([HD + 1, 1024], F32, tag="avps")
                avB = av_ps.tile([HD + 1, 1024], F32, tag="avps")
                av_tiles[hp] = (avA, avB)
            avA, avB = av_tiles[hp]
            for qc in range(2):
                nc.tensor.matmul(
                    avA[:, qc * 512:(qc + 1) * 512],
                    lhsT=V_AUG[:, kt, 2 * hp, :],
                    rhs=eA[:, qc * 512:(qc + 1) * 512],
                    start=(kt == 0), stop=(kt == TT - 1))
                nc.tensor.matmul(
                    avB[:, qc * 512:(qc + 1) * 512],
                    lhsT=V_AUG[:, kt, 2 * hp + 1, :],
                    rhs=eB[:, qc * 512:(qc + 1) * 512],
                    start=(kt == 0), stop=(kt == TT - 1))

        def emit_norm(hp):
            # outT[poff.., hp, :] = V-rows * (1/D); av row 0 is D, rows
            # 1..HD the unnormalized AV. All on-chip: broadcast D across
            # partitions (GpSimd), reciprocal at 64 lanes (DVE), multiply.
            avA, avB = av_tiles.pop(hp)
            for av, poff in ((avA, 0), (avB, 64)):
                U = norm_pool.tile([HD + 1, 1024], F32, tag="U")
                nc.vector.tensor_copy(U[:], av[:])
                # HW partition_broadcast reads physical partition 0, so stage
                # the D row into a partition-0 tile first
                Drow = norm_pool.tile([1, 1024], F32, tag="Drow")
                nc.vector.tensor_copy(Drow[:], U[HD:HD + 1, :])
                bcD = norm_pool.tile([64, 1024], F32, tag="bcD")
                nc.gpsimd.partition_broadcast(bcD[:], Drow[:])
                bc = norm_pool.tile([64, 1024], F32, tag="bc")
                scr = norm_pool.tile([64, 1024], F32, tag="scr")
                nc.vector.reciprocal_approx_accurate(bc[:], bcD[:], scr[:])
                for qc in range(2):
                    nc.vector.tensor_mul(
                        outT[poff:poff + 64, hp, qc * 512:(qc + 1) * 512],
                        U[0:HD, qc * 512:(qc + 1) * 512],
                        bc[:, qc * 512:(qc + 1) * 512])

        # ---- head-pair-pipelined rounds ----
        # QK0 first (gated on the first two wqk DMAs + xt) so exp starts
        # ASAP; V fills hp0's rounds, next head pair's QK fills hp>=1.
        for part in range(4):
            emit_qk(0, part)
        avq = []           # (hp, kt, eA, eB) awaiting AV emission
        DELAY = 3
        pending_norms = []
        for hp in range(CT):
            for kt in range(TT):
                eA, eB = emit_scores(hp, kt)
                avq.append((hp, kt, eA, eB))
                if hp == 0:
                    emit_v(kt, 0)
                    emit_v(kt, 1)
                    if kt % 2 == 1:
                        emit_qk(1, kt // 2)
                elif hp < CT - 1 and kt % 2 == (hp & 1):
                    emit_qk(hp + 1, kt // 2)
                while len(avq) > DELAY:
                    h0, k0, e0, e1 = avq.pop(0)
                    emit_av(h0, k0, e0, e1)
                    if k0 == TT - 1:
                        emit_norm(h0)
        for h0, k0, e0, e1 in avq:
            emit_av(h0, k0, e0, e1)
            if k0 == TT - 1:
                pending_norms.append(h0)

        # ---- proj, split so ct 0..4 runs while the last head pair's norm
        # chain completes; only the ct=5 contribution trails it ----
        for h0 in pending_norms:
            emit_norm(h0)
        # scalar engine (idle once exps finish) evacuates the partials so the
        # scps PSUM ring isn't gated on the DVE, which is busy with the last
        # head pair's normalization at this point
        partials = []
        for tt in range(TT):
            pp = outsb_pool.tile([128, C], F32, tag="projp", bufs=TT)
            partials.append(pp)
            for nch in range(2):
                ps = sc_ps.tile([128, 384], F32, tag="scps")
                for ct in range(CT - 1):
                    nc.tensor.matmul(
                        ps[:],
                        lhsT=outT[:, ct, tt * 128:(tt + 1) * 128],
                        rhs=PW[:, ct, nch * 384:(nch + 1) * 384],
                        start=(ct == 0), stop=(ct == CT - 2))
                nc.scalar.activation(pp[:, nch * 384:(nch + 1) * 384], ps[:],
                                     mybir.ActivationFunctionType.Copy)
        for tt in range(TT):
            osb = outsb_pool.tile([128, C], F32, tag="outsb")
            for nch in range(2):
                ps = sc_ps.tile([128, 384], F32, tag="scps")
                nc.tensor.matmul(
                    ps[:],
                    lhsT=outT[:, CT - 1, tt * 128:(tt + 1) * 128],
                    rhs=PW[:, CT - 1, nch * 384:(nch + 1) * 384],
                    start=True, stop=True)
                sl = slice(nch * 384, (nch + 1) * 384)
                nc.vector.tensor_add(osb[:, sl], ps[:], pbb[:, sl])
                nc.vector.tensor_add(osb[:, sl], osb[:, sl],
                                     partials[tt][:, sl])
            nc.scalar.dma_start(out[tt * 128:(tt + 1) * 128, :], osb[:])


_CACHE = {}


def _get_runner():
    """Build + compile once; return a callable(in_maps) -> list of out dicts."""
    if "runner" in _CACHE:
        return _CACHE["runner"]

    import jax
    from jax.experimental.shard_map import shard_map
    from jax.sharding import Mesh, PartitionSpec
    from concourse import bass2jax

    nc = _build()
    bass2jax.install_neuronx_cc_hook()

    partition_name = (nc.partition_id_tensor.name if nc.partition_id_tensor
                      else None)
    in_names, out_names, out_avals, zero_outs = [], [], [], []
    for alloc in nc.m.functions[0].allocations:
        if not isinstance(alloc, mybir.MemoryLocationSet):
            continue
        name = alloc.memorylocations[0].name
        if alloc.kind == "ExternalInput":
            if name != partition_name:
                in_names.append(name)
        elif alloc.kind == "ExternalOutput":
            out_names.append(name)
            shape = tuple(alloc.tensor_shape)
            dtype = mybir.dt.np(alloc.dtype)
            out_avals.append(jax.core.ShapedArray(shape, dtype))
            zero_outs.append(np.zeros(shape, dtype))
    n_params = len(in_names)
    n_outs = len(out_avals)
    all_in_names = list(in_names) + list(out_names)
    if partition_name is not None:
        all_in_names.append(partition_name)
    donate = tuple(range(n_params, n_params + n_outs))

    def _body(*args):
        operands = list(args)
        if partition_name is not None:
            operands.append(bass2jax.partition_id_tensor())
        outs = bass2jax._bass_exec_p.bind(
            *operands,
            out_avals=tuple(out_avals),
            in_names=tuple(all_in_names),
            out_names=tuple(out_names),
            lowering_input_output_aliases=(),
            sim_require_finite=True,
            sim_require_nnan=True,
            nc=nc,
        )
        return tuple(outs)

    devices = jax.devices()[:N_CORES]
    mesh = Mesh(np.asarray(devices), ("core",))
    in_specs = (PartitionSpec("core"),) * (n_params + n_outs)
    out_specs = (PartitionSpec("core"),) * n_outs
    sharded = jax.jit(
        shard_map(_body, mesh=mesh, in_specs=in_specs, out_specs=out_specs,
                  check_rep=False),
        donate_argnums=donate, keep_unused=True)

    def runner(in_maps):
        concat_in = [
            np.concatenate([np.asarray(m[name]) for m in in_maps], axis=0)
            for name in in_names
        ]
        concat_zeros = [
            np.zeros((N_CORES * z.shape[0], *z.shape[1:]), z.dtype)
            for z in zero_outs
        ]
        out_arrs = sharded(*concat_in, *concat_zeros)
        return [
            {name: np.asarray(out_arrs[i]).reshape(N_CORES, *out_avals[i].shape)[c]
             for i, name in enumerate(out_names)}
            for c in range(N_CORES)
        ]

    _CACHE["runner"] = runner
    _CACHE["nc"] = nc
    return runner


def make_in_maps(x, qkv_w, proj_w, proj_b):
    import ml_dtypes
    bf16 = ml_dtypes.bfloat16
    x = np.asarray(x, dtype=np.float32)
    qkv_w = np.asarray(qkv_w, dtype=np.float32)
    proj_w = np.asarray(proj_w, dtype=np.float32)
    pb = np.asarray(proj_b, dtype=np.float32).reshape(1, C)

    # wqk[p, i, ct, j] = qkv_w[ct*128+p, ft*128+j] with blocks interleaved
    # [Q0, K0, Q1, K1, ...] so the kernel's first DMA covers QK0 exactly
    ft_order = [b for hp in range(CT) for b in (hp, hp + CT)]
    wqk = np.ascontiguousarray(
        qkv_w[:, :2 * C].reshape(CT, 128, 2 * CT, 128).transpose(1, 2, 0, 3)
        [:, ft_order]).astype(bf16)
    # wv[p, ct, f] = qkv_w[ct*128+p, 1536+f]
    wv = np.ascontiguousarray(
        qkv_w[:, 2 * C:].reshape(CT, 128, C).transpose(1, 0, 2)).astype(bf16)
    # pw[p, ct, f] = proj_w[ct*128+p, f]
    pw = np.ascontiguousarray(
        proj_w.reshape(CT, 128, C).transpose(1, 0, 2)).astype(bf16)

    maps = []
    for b in range(N_CORES):
        # xt[p, tt, ct, j] = x[b, tt*128+j, ct*128+p]
        xtb = np.ascontiguousarray(
            np.asarray(x[b]).reshape(TT, 128, CT, 128).transpose(3, 0, 2, 1)
        ).astype(bf16)
        maps.append({"xt": xtb, "wqk": wqk, "wv": wv, "pw": pw, "proj_b": pb})
    return maps


def kernel(x, qkv_w, proj_w, proj_b):
    runner = _get_runner()
    results = runner(make_in_maps(x, qkv_w, proj_w, proj_b))
    return np.stack([results[b]["out"] for b in range(N_CORES)], axis=0)
